# revision 42
# baseline (speedup 1.0000x reference)
"""4-bit quant linear (dense_mlp) on 8 TRN2 NeuronCores.

out[b,o] = sum_i x[b,i] * (scales[o]*q[i,o] - zeros[o]) + bias[o]
         = (x @ (scales*q))[b,o] + 1*bias[o] + rowsum(x)[b]*(-zeros[o])

q[r*8+k, o] = (qweight[r,o] >> 4k) & 0xF  (int4 nibbles, exact in bf16)

Per core (2D shard: tokens 4-way x outfeatures 2-way):
  - DVE unpacks qweight nibbles into a resident bf16 weight tensor
    W[i', o] = scales[o]*q[i,o] with a per-K-tile permutation of i
    (i = 1024r + 8j + k stored at i' = 128*(8r+k) + j); the x side
    applies the same permutation, so the contraction is consistent.
    Unpack is emitted ob-major so the PE can start on o-block 0 early.
  - ScalarE converts x fp32->bf16 (permuted) with accum_out row sums;
    bf16 x goes to a DRAM staging tile, and X-bar DMA transpose
    produces xT K-tiles [128 i, group b] - the PE does (almost) only
    matmuls.
  - Per (128b x 512o) block: 32 accumulating bf16 matmuls + one K=2
    "affine" matmul with lhsT=[ones; rowsum_row] and rhs=[bias; -zeros]
    that adds both the bias and the zero-point term inside PSUM.
    ScalarE copies psum->sbuf, DMA out.  (rowsum columns are turned
    into rows with tiny PE transposes, one per 128 tokens.)
  - o-blocks are processed in two phases (2 o-blocks each) with the
    transpose DMAs re-issued per phase, overlapping the tail of the
    weight unpack with the first phase's matmuls.
"""

import sys

if "/opt/trn_rl_repo" not in sys.path:
    sys.path.insert(0, "/opt/trn_rl_repo")

import numpy as np

import concourse.bass as bass
import concourse.tile as tile
from concourse import bacc, mybir
from concourse.masks import make_identity

B, S, IN, OUT = 4, 2048, 4096, 4096
PACK = 8
M_TOT = B * S
M_SPLIT, O_SPLIT = 4, 2  # 8 cores = 4 token-shards x 2 outfeature-shards
M_SH, O_SH = M_TOT // M_SPLIT, OUT // O_SPLIT
N_CORES = 8

P = 128  # partitions
NB = 512  # o-block (one PSUM bank of fp32)
XC = 1024  # x chunk (i per qweight row-tile: 128 rows * 8 nibbles)
BTG = 4  # b-tiles per group
NPH = 2  # o-phases (transpose re-issue granularity)

FP32 = mybir.dt.float32
BF16 = mybir.dt.bfloat16
INT32 = mybir.dt.int32
Alu = mybir.AluOpType
ACT_COPY = mybir.ActivationFunctionType.Copy


def build_kernel(
    m_sh=M_SH,
    o_sh=O_SH,
    in_dim=IN,
    use_dma_transpose=True,
    bench_iters=1,
    bench_variant="full",
    nph=1,
    affine=False,
    unpack_xc=True,
):
    assert in_dim % XC == 0 and m_sh % P == 0 and o_sh % NB == 0
    n_kt = in_dim // P  # K-tiles
    n_r = in_dim // XC  # qweight row-tiles (128 rows each)
    n_bt = m_sh // P  # token tiles
    n_ob = o_sh // NB  # o-blocks
    btg = min(BTG, n_bt)
    assert n_bt % btg == 0
    n_g = n_bt // btg
    nph = nph if n_ob % nph == 0 else 1

    nc = bacc.Bacc(
        "TRN2",
        target_bir_lowering=False,
        debug=False,
        enable_asserts=False,
    )
    x_d = nc.dram_tensor("x", [m_sh, in_dim], FP32, kind="ExternalInput").ap()
    qw_d = nc.dram_tensor(
        "qweight", [in_dim // PACK, o_sh], INT32, kind="ExternalInput"
    ).ap()
    scales_d = nc.dram_tensor("scales", [1, o_sh], FP32, kind="ExternalInput").ap()
    zeros_d = nc.dram_tensor("zeros", [1, o_sh], FP32, kind="ExternalInput").ap()
    bias_d = nc.dram_tensor("bias", [1, o_sh], FP32, kind="ExternalInput").ap()
    out_d = nc.dram_tensor("out", [m_sh, o_sh], FP32, kind="ExternalOutput").ap()

    def bcast_ap(src, parts=P):
        return bass.AP(
            tensor=src.tensor, offset=src.offset, ap=[[0, parts]] + src.ap[1:]
        )

    old_path = (not use_dma_transpose) or bench_variant in ("mmonly", "samew")

    with tile.TileContext(nc) as tc:
        with (
            tc.tile_pool(name="consts", bufs=1) as consts,
            tc.tile_pool(name="wpool", bufs=1) as wpool,
            tc.tile_pool(name="qwp", bufs=2) as qwp,
            tc.tile_pool(name="nibp", bufs=2 if (affine and not old_path) else 1) as nibp,
            tc.tile_pool(name="xp", bufs=2) as xp,
            tc.tile_pool(name="xbp", bufs=2) as xbp,
            tc.tile_pool(name="xtp", bufs=n_kt if use_dma_transpose else btg) as xtp,
            tc.tile_pool(name="rsp", bufs=2 * btg) as rsp,
            tc.tile_pool(name="outp", bufs=3 if (affine and not old_path) else 2) as outp,
            tc.tile_pool(name="pst", bufs=3, space="PSUM") as pst,
            tc.tile_pool(name="psm", bufs=4, space="PSUM") as psm,
            tc.tile_pool(name="xbfp", bufs=max(2, n_g), space="DRAM") as xbfp,
        ):
            # ---- constants ----
            identity = consts.tile([P, P], BF16)
            make_identity(nc, identity)
            scales_b = consts.tile([P, o_sh], BF16)
            nc.gpsimd.dma_start(out=scales_b, in_=bcast_ap(scales_d))
            dummy = consts.tile([P, 64], FP32)
            # biasnz[0,:] = bias, biasnz[1,:] = -zeros (rhs of the K=2
            # affine matmul appended to each accumulation group).
            # Engine ops can't start at partition 1, so: fill both rows
            # with zeros, negate the whole tile, then DMA bias over row 0.
            biasnz = None
            if not old_path and affine:
                biasnz = consts.tile([2, o_sh], BF16)
                nc.gpsimd.dma_start(out=biasnz, in_=bcast_ap(zeros_d, parts=2))
                nc.vector.tensor_scalar(
                    biasnz, biasnz, -1.0, None, op0=Alu.mult
                )
                nc.gpsimd.dma_start(
                    out=biasnz[0:1, :], in_=bcast_ap(bias_d, parts=1)
                )
            ones_row = nzeros_b = bias_row = None
            if old_path or not affine:
                ones_row = consts.tile([1, P], BF16)
                nc.vector.memset(ones_row, 1.0)
                nzeros_b = consts.tile([P, o_sh], BF16)
                nc.gpsimd.dma_start(out=nzeros_b, in_=bcast_ap(zeros_d))
                nc.vector.tensor_scalar(
                    nzeros_b, nzeros_b, -1.0, None, op0=Alu.mult
                )
                bias_row = consts.tile([1, o_sh], BF16)
                nc.gpsimd.dma_start(out=bias_row, in_=bcast_ap(bias_d, parts=1))

            pools = dict(
                qwp=qwp, nibp=nibp, xp=xp, xbp=xbp, xtp=xtp, rsp=rsp,
                outp=outp, pst=pst, psm=psm, xbfp=xbfp,
            )
            cfg = dict(
                n_kt=n_kt, n_r=n_r, n_bt=n_bt, n_ob=n_ob, btg=btg, n_g=n_g,
                nph=nph, o_sh=o_sh, use_dma_transpose=use_dma_transpose,
                variant=bench_variant, affine=affine, unpack_xc=unpack_xc,
                skip_unpack=bench_variant == "nounpack",
            )
            tens = dict(
                identity=identity, ones_row=ones_row, scales_b=scales_b,
                nzeros_b=nzeros_b, bias_row=bias_row, dummy=dummy,
                biasnz=biasnz, x_d=x_d, qw_d=qw_d, out_d=out_d,
            )
            w_sb = wpool.tile([P, n_kt * o_sh], BF16)
            body = _pass_body_old if old_path else _pass_body
            if bench_iters > 1:
                with tc.For_i(0, bench_iters, 1):
                    body(nc, pools, cfg, tens, w_sb)
            else:
                body(nc, pools, cfg, tens, w_sb)
    nc.compile()
    return nc


def _pass_body(nc, pools, cfg, tens, w_sb):
    """DMA-transpose path: PE does only matmuls + tiny rowsum transposes."""
    qwp, nibp, xp, xbp = pools["qwp"], pools["nibp"], pools["xp"], pools["xbp"]
    xtp, rsp, outp = pools["xtp"], pools["rsp"], pools["outp"]
    pst, psm, xbfp = pools["pst"], pools["psm"], pools["xbfp"]
    n_kt, n_r, n_bt, n_ob = cfg["n_kt"], cfg["n_r"], cfg["n_bt"], cfg["n_ob"]
    btg, n_g, nph, o_sh = cfg["btg"], cfg["n_g"], cfg["nph"], cfg["o_sh"]
    identity, scales_b, dummy = tens["identity"], tens["scales_b"], tens["dummy"]
    biasnz = tens["biasnz"]
    x_d, qw_d, out_d = tens["x_d"], tens["qw_d"], tens["out_d"]
    in_dim = n_r * XC

    # ---- unpack (ob-major so PE can start after o-block 0) ----
    UW = XC if cfg.get("unpack_xc") else NB
    if cfg.get("skip_unpack"):
        nc.vector.memset(w_sb[:, 0:XC], 0.01)
    for ob in range(0 if cfg.get("skip_unpack") else o_sh // UW):
        osl = bass.ds(ob * UW, UW)
        for r in range(n_r):
            qw_t = qwp.tile([P, UW], INT32, name="qw_t")
            nc.sync.dma_start(out=qw_t, in_=qw_d[r * P : (r + 1) * P, osl])
            for k in range(PACK):
                kp = r * PACK + k
                nib = nibp.tile([P, UW], INT32, name="nib")
                nc.vector.tensor_scalar(
                    nib,
                    qw_t,
                    4 * k,
                    0xF,
                    op0=Alu.logical_shift_right,
                    op1=Alu.bitwise_and,
                )
                nc.vector.tensor_tensor(
                    w_sb[:, bass.ds(kp * o_sh + ob * UW, UW)],
                    nib,
                    scales_b[:, osl],
                    op=Alu.mult,
                )

    xbfs = [None] * n_g
    lhs2s = [[None] * btg for _ in range(n_g)]
    obs_per_ph = n_ob // nph

    for h in range(nph):
        for g in range(n_g):
            if h == 0:
                # ---- x pipeline: load, convert(+rowsum), stage to DRAM ----
                xbf_g = xbfp.tile([btg * P, in_dim], BF16, name="xbf")
                xbfs[g] = xbf_g
                for bi in range(btg):
                    bt = g * btg + bi
                    bsl = slice(bt * P, (bt + 1) * P)
                    rs_part = rsp.tile(
                        [P, n_r], FP32, name="rs_part", bufs=btg + 2
                    )
                    for r in range(n_r):
                        x_t = xp.tile([P, XC], FP32, name="x_t")
                        nc.sync.dma_start(
                            out=x_t, in_=x_d[bsl, r * XC : (r + 1) * XC]
                        )
                        # permuted (j e) -> (e j) bf16 convert + partial sum
                        x_b = xbp.tile([P, XC], BF16, name="x_b")
                        nc.scalar.activation(
                            x_b.rearrange("p (e j) -> p e j", e=PACK),
                            x_t.rearrange("p (j e) -> p e j", e=PACK),
                            ACT_COPY,
                            scale=1.0,
                            accum_out=rs_part[:, r : r + 1],
                        )
                        nc.sync.dma_start(
                            out=xbf_g[
                                bi * P : (bi + 1) * P, r * XC : (r + 1) * XC
                            ],
                            in_=x_b,
                        )
                    rs_t = rsp.tile([P, 1], FP32, name="rs", bufs=n_bt + 2)
                    nc.scalar.activation(
                        dummy[:, :n_r],
                        rs_part,
                        ACT_COPY,
                        scale=1.0,
                        accum_out=rs_t,
                    )
                    if cfg["affine"]:
                        # rowsum column -> [2,128] row pair via one PE
                        # transpose of [ones | rowsum] columns
                        rs2 = rsp.tile([P, 2], BF16, name="rs2", bufs=btg + 2)
                        nc.gpsimd.memset(rs2[:, 0:1], 1.0)
                        nc.scalar.copy(out=rs2[:, 1:2], in_=rs_t)
                        lhs2 = rsp.tile([2, P], BF16, name="lhs2", bufs=n_bt + 2)
                        ps_r = pst.tile([2, P], BF16, name="ps_r")
                        nc.tensor.transpose(ps_r, rs2, identity)
                        nc.scalar.copy(out=lhs2, in_=ps_r)
                        lhs2s[g][bi] = lhs2
                    else:
                        lhs2s[g][bi] = rs_t

            # ---- xT K-tiles for this group via X-bar DMA transpose ----
            xt_ks = []
            for kp in range(n_kt):
                xt_k = xtp.tile([P, btg * P], BF16, name="xt")
                nc.scalar.dma_start(
                    out=xt_k,
                    in_=xbfs[g][:, kp * P : (kp + 1) * P],
                    transpose=True,
                )
                xt_ks.append(xt_k)

            # ---- matmul blocks for this phase's o-blocks ----
            if cfg["affine"]:
                for ob in range(h * obs_per_ph, (h + 1) * obs_per_ph):
                    osl = bass.ds(ob * NB, NB)
                    for bi in range(btg):
                        bt = g * btg + bi
                        ps = psm.tile([P, NB], FP32, name="ps")
                        for kp in range(n_kt):
                            nc.tensor.matmul(
                                ps,
                                lhsT=xt_ks[kp][:, bass.ds(bi * P, P)],
                                rhs=w_sb[:, bass.ds(kp * o_sh + ob * NB, NB)],
                                start=(kp == 0),
                                stop=False,
                            )
                        # += 1*bias[o] + rowsum[b]*(-zeros[o])
                        nc.tensor.matmul(
                            ps,
                            lhsT=lhs2s[g][bi],
                            rhs=biasnz[:, osl],
                            start=False,
                            stop=True,
                        )
                        o_t = outp.tile([P, NB], FP32, name="o_t")
                        nc.scalar.copy(out=o_t, in_=ps)
                        nc.sync.dma_start(
                            out=out_d[bt * P : (bt + 1) * P, osl], in_=o_t
                        )
            else:
                ogrp = 2 if obs_per_ph % 2 == 0 else 1
                ones_row, bias_row = tens["ones_row"], tens["bias_row"]
                nzeros_b = tens["nzeros_b"]
                for og in range(h * obs_per_ph // ogrp, (h + 1) * obs_per_ph // ogrp):
                    for bi in range(btg):
                        bt = g * btg + bi
                        o_t = outp.tile([P, ogrp * NB], FP32, name="o_t")
                        for oj in range(ogrp):
                            ob = og * ogrp + oj
                            osl = bass.ds(ob * NB, NB)
                            ps = psm.tile([P, NB], FP32, name="ps")
                            for kp in range(n_kt):
                                nc.tensor.matmul(
                                    ps,
                                    lhsT=xt_ks[kp][:, bass.ds(bi * P, P)],
                                    rhs=w_sb[:, bass.ds(kp * o_sh + ob * NB, NB)],
                                    start=(kp == 0),
                                    stop=False,
                                )
                            nc.tensor.matmul(
                                ps, lhsT=ones_row, rhs=bias_row[:, osl],
                                start=False, stop=True,
                            )
                            nc.scalar.copy(
                                out=o_t[:, bass.ds(oj * NB, NB)], in_=ps
                            )
                        ogsl = bass.ds(og * ogrp * NB, ogrp * NB)
                        nc.vector.scalar_tensor_tensor(
                            o_t, nzeros_b[:, ogsl], lhs2s[g][bi], o_t,
                            op0=Alu.mult, op1=Alu.add,
                        )
                        nc.sync.dma_start(
                            out=out_d[bt * P : (bt + 1) * P, ogsl], in_=o_t
                        )


def _pass_body_old(nc, pools, cfg, tens, w_sb):
    """PE-transpose path + timing probe variants."""
    qwp, nibp, xp, xbp = pools["qwp"], pools["nibp"], pools["xp"], pools["xbp"]
    xtp, rsp, outp = pools["xtp"], pools["rsp"], pools["outp"]
    pst, psm, xbfp = pools["pst"], pools["psm"], pools["xbfp"]
    n_kt, n_r, n_bt, n_ob = cfg["n_kt"], cfg["n_r"], cfg["n_bt"], cfg["n_ob"]
    btg, o_sh = cfg["btg"], cfg["o_sh"]
    use_dma_transpose = cfg["use_dma_transpose"]
    variant = cfg.get("variant", "full")
    identity, ones_row = tens["identity"], tens["ones_row"]
    scales_b, nzeros_b = tens["scales_b"], tens["nzeros_b"]
    bias_row, dummy = tens["bias_row"], tens["dummy"]
    x_d, qw_d, out_d = tens["x_d"], tens["qw_d"], tens["out_d"]
    ogrp = 2 if n_ob % 2 == 0 else 1

    if variant in ("mmonly", "samew"):
        nc.vector.memset(w_sb[:, 0:XC], 1.0)
        xt_ks = []
        for kp in range(n_kt):
            xt_k = xtp.tile([P, btg * P], BF16, name="xt")
            nc.gpsimd.memset(xt_k, 0.5)
            xt_ks.append(xt_k)
        rs_t = rsp.tile([P, 1], FP32, name="rs", bufs=btg + 2)
        nc.vector.memset(rs_t, 1.0)
        for g in range(n_bt // btg):
            for og in range(n_ob // ogrp):
                for bi in range(btg):
                    bt = g * btg + bi
                    o_t = outp.tile([P, ogrp * NB], FP32, name="o_t")
                    for oj in range(ogrp):
                        ob = og * ogrp + oj
                        ps = psm.tile([P, NB], FP32, name="ps")
                        for kp in range(n_kt):
                            lhs = (
                                xt_ks[0][:, 0:P]
                                if variant == "samew"
                                else xt_ks[kp][:, bass.ds(bi * P, P)]
                            )
                            nc.tensor.matmul(
                                ps,
                                lhsT=lhs,
                                rhs=w_sb[:, bass.ds(ob * NB, NB)],
                                start=(kp == 0),
                                stop=(kp == n_kt - 1),
                            )
                        nc.scalar.copy(out=o_t[:, bass.ds(oj * NB, NB)], in_=ps)
                    ogsl = bass.ds(og * ogrp * NB, ogrp * NB)
                    nc.vector.scalar_tensor_tensor(
                        o_t, nzeros_b[:, ogsl], rs_t, o_t,
                        op0=Alu.mult, op1=Alu.add,
                    )
                    nc.sync.dma_start(
                        out=out_d[bt * P : (bt + 1) * P, ogsl], in_=o_t
                    )
        return

    # ---- unpack weights (XC-wide) ----
    for ob2 in range(o_sh // XC):
        osl = bass.ds(ob2 * XC, XC)
        for r in range(n_r):
            qw_t = qwp.tile([P, XC], INT32, name="qw_t")
            nc.sync.dma_start(out=qw_t, in_=qw_d[r * P : (r + 1) * P, osl])
            for k in range(PACK):
                kp = r * PACK + k
                nib = nibp.tile([P, XC], INT32, name="nib")
                nc.vector.tensor_scalar(
                    nib, qw_t, 4 * k, 0xF,
                    op0=Alu.logical_shift_right, op1=Alu.bitwise_and,
                )
                nc.vector.tensor_tensor(
                    w_sb[:, bass.ds(kp * o_sh + ob2 * XC, XC)],
                    nib,
                    scales_b[:, osl],
                    op=Alu.mult,
                )

    for g in range(n_bt // btg):
        xts, rss = [], []
        for bi in range(btg):
            bt = g * btg + bi
            bsl = slice(bt * P, (bt + 1) * P)
            xt_t = xtp.tile([P, n_kt * P], BF16, name="xt")
            rs_part = rsp.tile([P, n_r], FP32, name="rs_part", bufs=btg + 2)
            for r in range(n_r):
                x_t = xp.tile([P, XC], FP32, name="x_t")
                nc.sync.dma_start(out=x_t, in_=x_d[bsl, r * XC : (r + 1) * XC])
                x_b = xbp.tile([P, XC], BF16, name="x_b")
                nc.scalar.activation(
                    x_b.rearrange("p (e j) -> p e j", e=PACK),
                    x_t.rearrange("p (j e) -> p e j", e=PACK),
                    ACT_COPY,
                    scale=1.0,
                    accum_out=rs_part[:, r : r + 1],
                )
                x_r = x_b.rearrange("p (e j) -> p e j", e=PACK)
                for k in range(PACK):
                    kp = r * PACK + k
                    ps_t = pst.tile([P, P], BF16, name="ps_t")
                    nc.tensor.transpose(ps_t, x_r[:, k, :], identity)
                    nc.scalar.copy(out=xt_t[:, bass.ds(kp * P, P)], in_=ps_t)
            rs_t = rsp.tile([P, 1], FP32, name="rs", bufs=btg + 2)
            nc.scalar.activation(
                dummy[:, :n_r], rs_part, ACT_COPY, scale=1.0, accum_out=rs_t
            )
            xts.append(xt_t)
            rss.append(rs_t)

        for og in range(n_ob // ogrp):
            for bi in range(btg):
                bt = g * btg + bi
                o_t = outp.tile([P, ogrp * NB], FP32, name="o_t")
                for oj in range(ogrp):
                    ob = og * ogrp + oj
                    osl = bass.ds(ob * NB, NB)
                    ps = psm.tile([P, NB], FP32, name="ps")
                    for kp in range(n_kt):
                        nc.tensor.matmul(
                            ps,
                            lhsT=xts[bi][:, bass.ds(kp * P, P)],
                            rhs=w_sb[:, bass.ds(kp * o_sh + ob * NB, NB)],
                            start=(kp == 0),
                            stop=False,
                        )
                    nc.tensor.matmul(
                        ps, lhsT=ones_row, rhs=bias_row[:, osl],
                        start=False, stop=True,
                    )
                    nc.scalar.copy(out=o_t[:, bass.ds(oj * NB, NB)], in_=ps)
                ogsl = bass.ds(og * ogrp * NB, ogrp * NB)
                nc.vector.scalar_tensor_tensor(
                    o_t, nzeros_b[:, ogsl], rss[bi], o_t,
                    op0=Alu.mult, op1=Alu.add,
                )
                nc.sync.dma_start(out=out_d[bt * P : (bt + 1) * P, ogsl], in_=o_t)


_nc_full = None


def _shard_inputs(x, qweight, scales, zeros, bias):
    x_flat = np.ascontiguousarray(x.reshape(M_TOT, IN), dtype=np.float32)
    in_maps = []
    for c in range(N_CORES):
        mb, ob = divmod(c, O_SPLIT)
        osl = slice(ob * O_SH, (ob + 1) * O_SH)
        in_maps.append(
            {
                "x": np.ascontiguousarray(x_flat[mb * M_SH : (mb + 1) * M_SH]),
                "qweight": np.ascontiguousarray(qweight[:, osl]),
                "scales": np.ascontiguousarray(
                    np.asarray(scales, dtype=np.float32).reshape(OUT)[osl][None, :]
                ),
                "zeros": np.ascontiguousarray(
                    np.asarray(zeros, dtype=np.float32).reshape(OUT)[osl][None, :]
                ),
                "bias": np.ascontiguousarray(
                    np.asarray(bias, dtype=np.float32).reshape(OUT)[osl][None, :]
                ),
            }
        )
    return in_maps


def kernel(x, qweight, scales, zeros, bias):
    global _nc_full
    from concourse import bass_utils

    if _nc_full is None:
        _nc_full = build_kernel()
    in_maps = _shard_inputs(
        np.asarray(x),
        np.asarray(qweight),
        np.asarray(scales),
        np.asarray(zeros),
        np.asarray(bias),
    )
    res = bass_utils.run_bass_kernel_spmd(
        _nc_full, in_maps, core_ids=list(range(N_CORES))
    )
    out = np.empty((M_TOT, OUT), np.float32)
    for c in range(N_CORES):
        mb, ob = divmod(c, O_SPLIT)
        out[mb * M_SH : (mb + 1) * M_SH, ob * O_SH : (ob + 1) * O_SH] = res.results[
            c
        ]["out"]
    return out.reshape(B, S, OUT)
